# revision 53
# baseline (speedup 1.0000x reference)
"""Multi-head attention (B=4, L=2048, D=512, H=8) on 8 Trainium2 cores.

Sharding: core c handles batch b = c//2, query rows [(c%2)*1024, +1024).
Fully local: each core projects the FULL K/V for its batch (no collectives,
no cross-core sync).

Key optimizations:
  * EVERY matmul runs in the PE's 128x128 array mode, so the array never
    reconfigures (mode switches drain the systolic pipe and cost ~160ns
    per matmul when 64-row score MMs alternate with 128-row attnV MMs).
    Scores have K=dk=64 only, so kT is stored as TWO zero-padded
    stationaries: kT_E (even head rows 0:64, zeros below) and kT_O
    (zeros above, odd head rows 64:128); the full-height qT moving
    operand hits zeros for the other head's rows.
  * The padding mask lives in V, not in exp: masked kv rows of V are
    zeroed (data AND the ones-denominator column), so their exp'd scores
    vanish from both the numerator and denominator. exp then needs no
    per-chunk bias (one [128,512] op per kv chunk on either engine).
  * exp(scores) is split across TWO engines: ACT computes exact Exp;
    DVE computes a Schraudolph bit-trick exp in ONE tensor_scalar
    (i16 = A16*s + B16, int16 bits reinterpreted as bf16). Softmax
    renormalization cancels the ~3% multiplicative error.
  * Attention runs qh-sequentially (one 512-query half per pass) so only
    two 1-bank PSUM accumulators are live, leaving 4 banks for a 4-chunk
    score pipeline (PE never waits on the exp engines).
  * Biases: Q/K fused into the PSUM evacuations (per-partition bias on
    ACT/DVE); bv is folded through Wo on the host (softmax weights sum
    to 1, so attn(v+bv) = attn(v) + bv, i.e. bo' = bv@Wo + bo) — the
    V projection needs no bias at all; bo' via scalar_tensor_tensor.
  * Inputs/weights host-packed so every DMA moves 4KB contiguous rows.

Per-core device layout:
  xq/xk/xv piece tiles (128,2048)    512-pos slice of all 4 dmodel chunks
  qT (128,1024)x4                    head h at chunk h//2, partitions 64*(h%2)
  kT_E/kT_O (128,2048)x4             zero-padded per head parity
  V  (128, 544)x16                   kv chunk tiles; head h cols [68h,68h+64)
                                     data, col 68h+64 = mask (denominator)
  ss (128,512) PSUM x5               scores [kv, q-half] (1 bank each)
  xs (65,512)  PSUM x3               attnV accum; row 64 = softmax denom
"""
import numpy as np
import ml_dtypes

import concourse.bacc as bacc
import concourse.bass as bass
import concourse.mybir as mybir
import concourse.tile as tile
from concourse.bass_utils import run_bass_kernel_spmd

F32 = mybir.dt.float32
BF16 = mybir.dt.bfloat16
I16 = mybir.dt.int16
AF = mybir.ActivationFunctionType
ALU = mybir.AluOpType

B, L, D = 4, 2048, 512
H, DK = 8, 64
N_CORES = 8
LQ = L // 2            # query rows per core
P = 128
KVC = L // P           # 16 kv chunks
MC = D // P            # 4 dmodel chunks
VW = 68                # per-head stride in V tiles (64 data + mask + pad)

# Schraudolph exp constants (bf16-bits variant): bf16_bits(exp(x)) ~=
# int16(A16*x + B16)
_A = 2.0 ** 23 / np.log(2.0)
_C = 486411.0
A16 = float(_A / 65536.0)
B16 = float((127.0 * 2.0 ** 23 - _C) / 65536.0)

MM_DT = BF16
MM_NP = ml_dtypes.bfloat16

_cache = {}


def _build():
    nc = bacc.Bacc("TRN2", target_bir_lowering=False, debug=False,
                   num_devices=N_CORES)

    # Inputs/weights are host-packed so every DMA moves whole 4KB DRAM rows:
    # piece tensors [n, 128, 4*512] hold one 512-col slice of all 4 chunks.
    xq_d = nc.dram_tensor("xq", [2, P, 2048], MM_DT, kind="ExternalInput").ap()
    xk_d = nc.dram_tensor("xk", [4, P, 2048], MM_DT, kind="ExternalInput").ap()
    xv_d = nc.dram_tensor("xv", [4, P, 2048], MM_DT, kind="ExternalInput").ap()
    wq_d = nc.dram_tensor("wq", [P, 2048], MM_DT, kind="ExternalInput").ap()
    wk_d = nc.dram_tensor("wk", [P, 2048], MM_DT, kind="ExternalInput").ap()
    wv_d = nc.dram_tensor("wv", [P, 2048], MM_DT, kind="ExternalInput").ap()
    wo_d = nc.dram_tensor("wo", [P, 2048], MM_DT, kind="ExternalInput").ap()
    bq_d = nc.dram_tensor("bq", [P, MC], F32, kind="ExternalInput").ap()
    bk_d = nc.dram_tensor("bk", [P, MC], F32, kind="ExternalInput").ap()
    bo_d = nc.dram_tensor("bo", [1, D], F32, kind="ExternalInput").ap()
    m01_d = nc.dram_tensor("m01", [P, KVC], F32, kind="ExternalInput").ap()
    m8_d = nc.dram_tensor("m8", [P, KVC * H], MM_DT,
                          kind="ExternalInput").ap()
    out_d = nc.dram_tensor("out", [LQ, D], F32, kind="ExternalOutput").ap()

    mm = nc.tensor.matmul

    def dma_in(t, src):
        nc.sync.dma_start(t, src)

    with tile.TileContext(nc) as tc:
        with tc.tile_pool(name="const", bufs=1) as cpool, \
             tc.tile_pool(name="xin", bufs=1) as xpool, \
             tc.tile_pool(name="proj", bufs=1) as prpool, \
             tc.tile_pool(name="attn", bufs=5) as apool, \
             tc.tile_pool(name="norm", bufs=2) as npool, \
             tc.tile_pool(name="outp", bufs=3) as opool, \
             tc.tile_pool(name="ps", bufs=1, space="PSUM") as ps:

            def load_packed(pool, ap3d, nm, n):
                tiles = []
                for p_ in range(n):
                    t = pool.tile([P, 2048], MM_DT, tag=f"{nm}{p_}",
                                  name=f"{nm}{p_}")
                    dma_in(t[:], ap3d[p_])
                    tiles.append(t)
                return tiles

            def load_w(pool, ap2d, nm):
                t = pool.tile([P, 2048], MM_DT, tag=nm, name=nm)
                dma_in(t[:], ap2d[:, :])
                return t

            def load_small(ap2d, nm):
                t = cpool.tile(list(ap2d.shape), ap2d.dtype, tag=nm, name=nm)
                dma_in(t[:], ap2d[:, :])
                return t

            # loads in first-use order; piece tile p_, chunk kc lives at
            # columns [kc*512, +512) (weights: [kc*512 + j])
            wq = load_w(cpool, wq_d, "wq")
            xqP = load_packed(xpool, xq_d, "xq", 2)
            bq = load_small(bq_d, "bq")
            wk = load_w(cpool, wk_d, "wk")
            bk = load_small(bk_d, "bk")
            xkP = load_packed(xpool, xk_d, "xk", 4)
            wv = load_w(cpool, wv_d, "wv")
            m01 = load_small(m01_d, "m01")
            m8 = load_small(m8_d, "m8")
            xvP = load_packed(xpool, xv_d, "xv", 4)
            wo = load_w(cpool, wo_d, "wo")
            bo = load_small(bo_d, "bo")
            wup = cpool.tile([P, 512], MM_DT, tag="wup", name="wup")
            nc.gpsimd.memset(wup[:], 0.0)
            for _ in range(3):
                wps = ps.tile([P, 512], F32, tag="ss", bufs=5, name="wps")
                for _k in range(8):
                    mm(wps[:], wup[:, 0:P], wup[:],
                       start=_k == 0, stop=_k == 7)
            bo_bc = cpool.tile([P, D], F32)
            nc.gpsimd.partition_broadcast(bo_bc[:], bo[:])

            def w_chunk(w, kc, lo, hi):
                return w[:, kc * 512 + lo:kc * 512 + hi]

            def ss_tile(name):
                return ps.tile([P, 512], F32, tag="ss", bufs=5, name=name)


            # ---- Q projection (4 MMs + 1 ACT bias/evac per [128,512]) ----
            qT = [prpool.tile([P, LQ], MM_DT, tag=f"qT{m}", name=f"qT{m}")
                  for m in range(MC)]
            for m in range(MC):
                for s in range(2):
                    pp = ss_tile("ppq")
                    for kc in range(MC):
                        mm(pp[:], w_chunk(wq, kc, m * P, (m + 1) * P),
                           xqP[s][:, kc * 512:(kc + 1) * 512],
                           start=kc == 0, stop=kc == MC - 1)
                    nc.scalar.activation(qT[m][:, s * 512:(s + 1) * 512],
                                         pp[:], AF.Identity,
                                         bias=bq[:, m:m + 1])

            # ---- K projection, zero-padded per head parity ----
            kT_E = [prpool.tile([P, L], MM_DT, tag=f"kTE{m}", name=f"kTE{m}")
                    for m in range(MC)]
            kT_O = [prpool.tile([P, L], MM_DT, tag=f"kTO{m}", name=f"kTO{m}")
                    for m in range(MC)]
            for m in range(MC):
                nc.gpsimd.memset(kT_E[m][64:128, :], 0.0)
                nc.gpsimd.memset(kT_O[m][0:64, :], 0.0)
            for m in range(MC):
                for s in range(4):
                    pp = ss_tile("ppk")
                    for kc in range(MC):
                        mm(pp[:], w_chunk(wk, kc, m * P, (m + 1) * P),
                           xkP[s][:, kc * 512:(kc + 1) * 512],
                           start=kc == 0, stop=kc == MC - 1)
                    sl = slice(s * 512, (s + 1) * 512)
                    nc.scalar.activation(kT_E[m][0:64, sl], pp[0:64, :],
                                         AF.Identity, bias=bk[0:64, m:m + 1])
                    nc.vector.tensor_scalar_add(kT_O[m][64:128, sl],
                                                pp[64:128, :],
                                                bk[64:128, m:m + 1])

            # ---- V projection: bias via K=1 ones MM, mask via evac mult ----
            v_sb = prpool.tile([P, KVC * VW * H], MM_DT, tag="V", name="v_sb")
            v_g = v_sb.rearrange("p (t h d) -> p t h d", t=KVC, d=VW)
            nc.gpsimd.memset(v_sb[:], 0.0)
            # mask column (the softmax denominator rides the 65th V column)
            nc.vector.tensor_copy(
                v_g[:, :, :, 64], m8[:].rearrange("p (t h) -> p t h", h=H))
            for t in range(KVC):
                pv = ss_tile("ppv")
                for kc in range(MC):
                    mm(pv[:], xvP[t // 4][:, kc * 512 + (t % 4) * P:
                                          kc * 512 + (t % 4 + 1) * P],
                       w_chunk(wv, kc, 0, 512), start=kc == 0,
                       stop=kc == MC - 1)
                nc.vector.tensor_scalar_mul(
                    v_g[:, t, :, 0:64],
                    pv.rearrange("p (h d) -> p h d", d=64),
                    m01[:, t:t + 1])

            def v_head(t, h):
                return v_g[:, t, h, 0:65]

            # ---- attention: qh-sequential passes, all MMs 128x128 mode ----
            xsT2 = [prpool.tile([P, LQ], MM_DT, tag=f"xs{hp}",
                                name=f"xsT2_{hp}")
                    for hp in range(MC)]

            def out_proj(qt):
                po = ss_tile("ppo")
                for hp in range(MC):
                    mm(po[:], xsT2[hp][:, qt * P:(qt + 1) * P],
                       w_chunk(wo, hp, 0, 512), start=hp == 0,
                       stop=hp == MC - 1)
                osb = opool.tile([P, 512], F32, tag="osb")
                nc.vector.scalar_tensor_tensor(osb[:], po[:], 1.0, bo_bc[:],
                                               ALU.mult, ALU.add)
                nc.sync.dma_start(out_d[qt * P:(qt + 1) * P, :], osb[:])

            for hp in range(MC):
                hE, hO = 2 * hp, 2 * hp + 1
                for qh in range(2):
                    xsE = ps.tile([65, 512], F32, tag="xs", bufs=3,
                                  name=f"xsE{hp}{qh}")
                    xsO = ps.tile([65, 512], F32, tag="xs", bufs=3,
                                  name=f"xsO{hp}{qh}")
                    at_tiles = {}
                    qsl = slice(qh * 512, (qh + 1) * 512)

                    def scores(c):
                        ssA = ss_tile("ssA")
                        ssB = ss_tile("ssB")
                        mm(ssA[:], kT_E[hp][:, c * P:(c + 1) * P],
                           qT[hp][:, qsl], start=True, stop=True)
                        mm(ssB[:], kT_O[hp][:, c * P:(c + 1) * P],
                           qT[hp][:, qsl], start=True, stop=True)
                        aE = apool.tile([P, 512], MM_DT, tag="atE",
                                        bufs=10, name="aE")
                        aO = apool.tile([P, 512], MM_DT, tag="atO",
                                        bufs=10, name="aO")
                        nc.scalar.activation(aE[:], ssA[:], AF.Exp,
                                             scale=0.125)
                        if c == 7 or (c == 12 and qh == 0):  # engine balance
                            nc.scalar.activation(aO[:], ssB[:], AF.Exp,
                                                 scale=0.125)
                        else:
                            nc.vector.tensor_scalar(
                                aO.bitcast(I16)[:], ssB[:], A16 * 0.125,
                                B16, ALU.mult, ALU.add)
                        at_tiles[c] = (aE, aO)

                    def attnv(c):
                        aE, aO = at_tiles.pop(c)
                        mm(xsE[:], v_head(c, hE), aE[:],
                           start=c == 0, stop=c == KVC - 1,
                           skip_group_check=True)
                        mm(xsO[:], v_head(c, hO), aO[:],
                           start=c == 0, stop=c == KVC - 1,
                           skip_group_check=True)

                    scores(0)
                    scores(1)
                    for c in range(2, KVC):
                        scores(c)
                        attnv(c - 2)
                    attnv(KVC - 2)
                    attnv(KVC - 1)

                    # normalize: xsT2 = xs[0:64] / xs[64] (denominator row)
                    # (approx-recip is a custom DVE op: stage the denominator
                    # row to SBUF first — reading PSUM there breaks on HW)
                    for par, xs_t in ((0, xsE), (1, xsO)):
                        srow = npool.tile([1, 512], F32, tag="srow")
                        nc.scalar.copy(srow[:], xs_t[64:65, :])
                        rec = npool.tile([1, 512], F32, tag="rec")
                        nc.vector.reciprocal_approx_fast(rec[:], srow[:])
                        bc = npool.tile([64, 512], F32, tag="bc")
                        nc.gpsimd.partition_broadcast(bc[:], rec[:])
                        nc.vector.tensor_tensor(
                            xsT2[hp][64 * par:64 * par + 64, qsl],
                            xs_t[0:64, :], bc[:], ALU.mult)

                    # overlap the output projection with the last pass
                    if hp == MC - 1:
                        for qt in range(qh * 4, qh * 4 + 4):
                            out_proj(qt)

    nc.compile()
    return nc


def _host_inputs(query, key, value, mask, Wq, bq, Wk, bk, Wv, bv, Wo, bo):
    """Build the 8 per-core input maps (all rank-dependence lives here)."""
    f32 = np.float32

    def pack_w(W):
        # [512, 512] -> [128, 2048]: chunk kc at columns [kc*512, +512)
        w = np.asarray(W).astype(MM_NP).reshape(MC, P, D)
        return np.ascontiguousarray(w.transpose(1, 0, 2).reshape(P, MC * D))

    def pack_x(x, n):
        # x [rows, 512] -> pieces [n, 128, 2048]: piece p_ holds the
        # 512-row slice p_ of x transposed, chunk kc at cols [kc*512,+512)
        xT = np.ascontiguousarray(x.T).astype(MM_NP)        # [512, rows]
        rows = xT.shape[1]
        pw = rows // n
        out = np.empty((n, P, MC * pw), MM_NP)
        for p_ in range(n):
            blk = xT[:, p_ * pw:(p_ + 1) * pw].reshape(MC, P, pw)
            out[p_] = blk.transpose(1, 0, 2).reshape(P, MC * pw)
        return out

    wq_ = pack_w(Wq)
    wk_ = pack_w(Wk)
    wv_ = pack_w(Wv)
    wo_ = pack_w(Wo)
    bq_ = np.ascontiguousarray(bq.astype(f32).reshape(MC, P).T)
    bk_ = np.ascontiguousarray(bk.astype(f32).reshape(MC, P).T)
    bo_ = (np.asarray(bv, f32) @ np.asarray(Wo, f32)
           + np.asarray(bo, f32)).reshape(1, D)
    in_maps = []
    for c in range(N_CORES):
        b, half = c // 2, c % 2
        sl = slice(half * LQ, (half + 1) * LQ)
        xq_ = pack_x(query[b, sl, :], 2)
        xk_ = pack_x(key[b], 4)
        xv_ = pack_x(value[b], 4)
        m01_ = np.ascontiguousarray(
            (mask[b] != 0).astype(f32).reshape(KVC, P).T)
        m8_ = np.ascontiguousarray(
            np.repeat(m01_.astype(MM_NP)[:, :, None], H, axis=2)
            .reshape(P, KVC * H))
        in_maps.append({
            "xq": xq_, "xk": xk_, "xv": xv_,
            "wq": wq_, "wk": wk_, "wv": wv_, "wo": wo_,
            "bq": bq_, "bk": bk_, "bo": bo_,
            "m01": m01_, "m8": m8_,
        })
    return in_maps


def kernel(query, key, value, mask, Wq, bq, Wk, bk, Wv, bv, Wo, bo):
    if "nc" not in _cache:
        _cache["nc"] = _build()
    nc = _cache["nc"]
    in_maps = _host_inputs(query, key, value, mask,
                           Wq, bq, Wk, bk, Wv, bv, Wo, bo)
    res = run_bass_kernel_spmd(nc, in_maps, list(range(N_CORES))).results
    out = np.empty((B, L, D), np.float32)
    for c in range(N_CORES):
        b, half = c // 2, c % 2
        out[b, half * LQ:(half + 1) * LQ, :] = res[c]["out"]
    return out
